# revision 54
# baseline (speedup 1.0000x reference)
"""GQA attention (RoPE + ALiBi + causal) on 8 trn2 NeuronCores.

Sharding: core c -> batch b = c//4, kv-group g = c%4 (4 q-heads + 1 kv-head
per core, column-sharded Wq/Wk/Wv, row-sharded Wo; host sums the 4 partial
Wo outputs per batch).

All matmuls run in bf16 (f32 PSUM accumulation). Everything is kept
transposed ([feature, token]) so softmax reductions over keys become
partition-dim reductions done with ones-vector matmuls, and the per-key
ALiBi column bias rides the exp() activation's per-partition bias (the
per-query ALiBi part is constant per token and cancels in softmax up to
the per-block offset baked into the bias). Causal structure: only
lower-triangle key tiles are computed; diagonal tiles are masked with a
multiplicative {0,1} mask after exp. V is projected directly transposed
(x-tile stationary). The emission order interleaves next-block
projections between attention and Wo so the PE never idles long enough
for HAM to re-throttle.
"""
import sys

if '/opt/trn_rl_repo' not in sys.path:
    sys.path.insert(0, '/opt/trn_rl_repo')

import numpy as np
import ml_dtypes

BFNP = ml_dtypes.bfloat16

B, T, D = 2, 2048, 2048
H, KV = 16, 4
HD = D // H          # 128
NREP = H // KV       # 4
KVD = 512            # per-core q width (4 heads x 128)
P = 128
TB = 512             # t-block
NBLK = T // TB       # 4
NC = D // P          # 16 contraction tiles
NJ = T // P          # 16 key tiles
ALIBI_W = 0.1
SCALE = (1.0 - ALIBI_W) / np.sqrt(np.float32(HD))

# stream_shuffle: rotate partitions by 64 (16 groups of 4) - self-inverse
SHUF_MASK = [(g + 16) % 32 for g in range(32)]

_cache = {}


def _build():
    from concourse import bacc, mybir, bass_isa
    from concourse.tile import TileContext

    F32 = mybir.dt.float32
    BF16 = mybir.dt.bfloat16
    EXP = mybir.ActivationFunctionType.Exp

    nc = bacc.Bacc()
    xT = nc.declare_dram_parameter("xT", [D, T], BF16, isOutput=False)
    wq = nc.declare_dram_parameter("wq", [D, KVD], BF16, isOutput=False)
    wk = nc.declare_dram_parameter("wk", [D, P], BF16, isOutput=False)
    wv = nc.declare_dram_parameter("wv", [D, P], BF16, isOutput=False)
    wo = nc.declare_dram_parameter("wo", [KVD, D], BF16, isOutput=False)
    cosq = nc.declare_dram_parameter("cosq", [P, T], BF16, isOutput=False)
    sinq = nc.declare_dram_parameter("sinq", [P, T], BF16, isOutput=False)
    cosk = nc.declare_dram_parameter("cosk", [P, T], BF16, isOutput=False)
    sink = nc.declare_dram_parameter("sink", [P, T], BF16, isOutput=False)
    cb = nc.declare_dram_parameter("cb", [P, NREP * NBLK * NJ], F32, isOutput=False)
    m01 = nc.declare_dram_parameter("m01", [P, 4 * TB], BF16, isOutput=False)
    onesc = nc.declare_dram_parameter("onesc", [P, 1], BF16, isOutput=False)
    idin = nc.declare_dram_parameter("idin", [P, P], BF16, isOutput=False)
    out = nc.declare_dram_parameter("out", [T, D], BF16, isOutput=True)

    with TileContext(nc) as tc:
        with (
            tc.tile_pool(name="const", bufs=1) as cpool,
            tc.tile_pool(name="kv", bufs=1) as kvpool,
            tc.tile_pool(name="xin", bufs=2) as xpool,
            tc.tile_pool(name="work", bufs=2) as wpool,
            tc.tile_pool(name="qt", bufs=8) as qpool,
            tc.tile_pool(name="pt", bufs=6) as ptpool,
            tc.tile_pool(name="ot", bufs=8) as opool,
            tc.tile_pool(name="ysb", bufs=2) as ypool,
            tc.tile_pool(name="small", bufs=2) as spool,
            tc.tile_pool(name="ps", bufs=1, space="PSUM") as pss,
        ):
            # ---- resident constants ----
            # DMA priority: x block 0 + Wq first (needed by the first matmuls),
            # then the rest in order of first use.
            xTr = xT.rearrange("(c p) t -> p c t", p=P)
            xt0 = xpool.tile([P, NC, TB], BF16, tag="xt", name="xt0")
            wq_sb = cpool.tile([P, NC, KVD], BF16)
            wq_r = wq.rearrange("(c p) n -> p c n", p=P)
            # interleave small x/Wq chunks so the first q-wave matmuls can
            # start as soon as the first chunks land; spread the issue cost
            # across engines (each dma_start costs ~0.6us on its issuing
            # engine, all of which are idle during the prologue)
            for c2 in range(8):
                nc.sync.dma_start(out=xt0[:, 2 * c2:2 * c2 + 2],
                                  in_=xTr[:, 2 * c2:2 * c2 + 2, 0:TB])
                nc.sync.dma_start(out=wq_sb[:, 2 * c2:2 * c2 + 2],
                                  in_=wq_r[:, 2 * c2:2 * c2 + 2])
            wk_sb = cpool.tile([P, NC, P], BF16)
            wk_r = wk.rearrange("(c p) n -> p c n", p=P)
            nc.sync.dma_start(out=wk_sb, in_=wk_r)
            wv_sb = cpool.tile([P, NC, P], BF16)
            wv_r = wv.rearrange("(c p) n -> p c n", p=P)
            nc.sync.dma_start(out=wv_sb, in_=wv_r)
            cq_sb = cpool.tile([P, T], BF16)
            nc.sync.dma_start(out=cq_sb, in_=cosq[:, :])
            sq_sb = cpool.tile([P, T], BF16)
            nc.sync.dma_start(out=sq_sb, in_=sinq[:, :])
            ck_sb = cpool.tile([P, T], BF16)
            nc.sync.dma_start(out=ck_sb, in_=cosk[:, :])
            sk_sb = cpool.tile([P, T], BF16)
            nc.sync.dma_start(out=sk_sb, in_=sink[:, :])
            cb_sb = cpool.tile([P, NREP * NBLK * NJ], F32)
            nc.sync.dma_start(out=cb_sb, in_=cb[:, :])
            m01_sb = cpool.tile([P, 4 * TB], BF16)
            nc.sync.dma_start(out=m01_sb, in_=m01[:, :])
            ones_sb = cpool.tile([P, 1], BF16)
            nc.sync.dma_start(out=ones_sb, in_=onesc[:, :])
            id_sb = cpool.tile([P, P], BF16)
            nc.sync.dma_start(out=id_sb, in_=idin[:, :])
            wo_sb = cpool.tile([P, NREP, D], BF16)
            wo_r = wo.rearrange("(h p) e -> p h e", p=P)
            for h in range(NREP):
                nc.sync.dma_start(out=wo_sb[:, h], in_=wo_r[:, h])

            kT_sb = kvpool.tile([P, T], BF16)        # roped K, [d, s]
            v_sb = kvpool.tile([P, NJ * P], BF16)    # V tiles, [s, j*d']

            qh_l = [[None] * NREP for _ in range(NBLK)]
            oh_l = [[None] * NREP for _ in range(NBLK)]

            def rope(dst, src_ps, cos_ap, sin_ap, nm):
                # muls/adds on GpSimd (otherwise idle) to unload the DVE;
                # the cross-partition shuffle must stay on the DVE
                raw = wpool.tile([P, TB], BF16, tag="raw", name=f"raw{nm}")
                nc.scalar.copy(raw, src_ps)
                swp = wpool.tile([P, TB], BF16, tag="swp", name=f"swp{nm}")
                nc.vector.stream_shuffle(swp, raw, SHUF_MASK)
                m1 = wpool.tile([P, TB], BF16, tag="m1", name=f"m1{nm}")
                nc.vector.tensor_mul(m1, raw, cos_ap)
                m2 = wpool.tile([P, TB], BF16, tag="m2", name=f"m2{nm}")
                nc.vector.tensor_mul(m2, swp, sin_ap)
                nc.vector.tensor_add(dst, m1, m2)

            def proj(bk):
                t0 = bk * TB
                if bk == 0:
                    xt = xt0
                else:
                    xt = xpool.tile([P, NC, TB], BF16, tag="xt", name=f"xt{bk}")
                    nc.sync.dma_start(out=xt[:, 0:8], in_=xTr[:, 0:8, t0:t0 + TB])
                    nc.sync.dma_start(out=xt[:, 8:16], in_=xTr[:, 8:16, t0:t0 + TB])
                # q waves (one PSUM tile each)
                for h in range(NREP):
                    qp = pss.tile([P, TB], F32, tag="work", bufs=2, name=f"qp{bk}_{h}")
                    for c in range(NC):
                        nc.tensor.matmul(qp, wq_sb[:, c, h * P:(h + 1) * P], xt[:, c],
                                         start=(c == 0), stop=(c == NC - 1))
                    qh = qpool.tile([P, TB], BF16, tag="qh", name=f"qh{bk}_{h}")
                    rope(qh, qp, cq_sb[:, t0:t0 + TB], sq_sb[:, t0:t0 + TB], f"q{bk}{h}")
                    qh_l[bk][h] = qh
                # k wave
                kp = pss.tile([P, TB], F32, tag="work", bufs=2, name=f"kp{bk}")
                for c in range(NC):
                    nc.tensor.matmul(kp, wk_sb[:, c], xt[:, c], start=(c == 0), stop=(c == NC - 1))
                rope(kT_sb[:, t0:t0 + TB], kp, ck_sb[:, t0:t0 + TB], sk_sb[:, t0:t0 + TB], f"k{bk}")
                # v wave: project normally ([d', tok], one N=512 accumulation),
                # then transpose the four 128-token slices on the DMA engines
                # (2-byte xbar transpose) - keeps the PE free of N=128 matmuls
                vp = pss.tile([P, TB], F32, tag="work", bufs=2, name=f"vp{bk}")
                for c in range(NC):
                    nc.tensor.matmul(vp, wv_sb[:, c], xt[:, c], start=(c == 0), stop=(c == NC - 1))
                vtmp = wpool.tile([P, TB], BF16, tag="vtmp", name=f"vtmp{bk}")
                nc.vector.tensor_copy(vtmp, vp)
                for ts_ in range(4):
                    nc.sync.dma_start_transpose(
                        out=v_sb[:, (4 * bk + ts_) * P:(4 * bk + ts_ + 1) * P],
                        in_=vtmp[:, ts_ * P:(ts_ + 1) * P])

            def attn(bk):
                nj = 4 * bk + 4
                for h in range(NREP):
                    ot_ps = pss.tile([P, TB], F32, tag="ot", bufs=2, name=f"ot{bk}_{h}")
                    acc = spool.tile([P, TB], BF16, tag="acc", name=f"acc{bk}_{h}")

                    def post(sp, j):
                        # diag key tile delta: tokens < 128*delta never attend
                        # to these keys -> compute only the [128*delta, TB)
                        # token range; the triangular mask only applies to the
                        # first 128 tokens of that range (the corner)
                        delta = j - 4 * bk
                        d0 = max(delta, 0) * P
                        col = (h * NBLK + bk) * NJ + j
                        pt = ptpool.tile([P, TB], BF16, tag="pt", name=f"pt{bk}_{h}_{j}")
                        nc.scalar.activation(pt[:, d0:TB], sp[:, d0:TB], EXP,
                                             bias=cb_sb[:, col:col + 1])
                        if delta >= 0:
                            pmc = ptpool.tile([P, P], BF16, tag="pmc", bufs=2,
                                              name=f"pmc{bk}_{h}_{j}")
                            nc.vector.tensor_mul(pmc, pt[:, d0:d0 + P], m01_sb[:, 0:P])
                            if j == 0:
                                nc.vector.tensor_copy(acc[:, 0:P], pmc)
                                nc.vector.tensor_copy(acc[:, P:TB], pt[:, P:TB])
                            else:
                                nc.vector.tensor_add(acc[:, d0:d0 + P], acc[:, d0:d0 + P], pmc)
                                if d0 + P < TB:
                                    nc.vector.tensor_add(acc[:, d0 + P:TB], acc[:, d0 + P:TB],
                                                         pt[:, d0 + P:TB])
                            nc.tensor.matmul(ot_ps[:, d0:d0 + P], v_sb[:, j * P:(j + 1) * P],
                                             pmc, start=(j == 0), stop=(j == nj - 1))
                            if d0 + P < TB:
                                nc.tensor.matmul(ot_ps[:, d0 + P:TB], v_sb[:, j * P:(j + 1) * P],
                                                 pt[:, d0 + P:TB], start=False, stop=False)
                        else:
                            if j == 0:
                                nc.vector.tensor_copy(acc, pt)
                            else:
                                nc.vector.tensor_add(acc, acc, pt)
                            nc.tensor.matmul(ot_ps, v_sb[:, j * P:(j + 1) * P], pt,
                                             start=(j == 0), stop=False)

                    pend = None
                    for j in range(nj):
                        sp = pss.tile([P, TB], F32, tag="s", bufs=3, name=f"sp{bk}_{h}_{j}")
                        d0 = max(j - 4 * bk, 0) * P
                        nc.tensor.matmul(sp[:, d0:TB], kT_sb[:, j * P:(j + 1) * P],
                                         qh_l[bk][h][:, d0:TB], start=True, stop=True)
                        if pend is not None:
                            post(*pend)
                        pend = (sp, j)
                    post(*pend)

                    cs_ps = pss.tile([1, TB], F32, tag="cs", bufs=1, name=f"cs{bk}_{h}")
                    nc.tensor.matmul(cs_ps, ones_sb, acc, start=True, stop=True)
                    rin = spool.tile([1, TB], F32, tag="rin", name=f"rin{bk}_{h}")
                    nc.vector.reciprocal_approx_fast(out=rin, in_=cs_ps)
                    rbc = spool.tile([P, TB], F32, tag="rbc", name=f"rbc{bk}_{h}")
                    nc.gpsimd.partition_broadcast(rbc, rin)
                    oh = opool.tile([P, TB], BF16, tag="oh", name=f"oh{bk}_{h}")
                    nc.vector.tensor_mul(oh, ot_ps, rbc)
                    oh_l[bk][h] = oh

            def wo_stage(bk):
                t0 = bk * TB
                for ts_ in range(4):
                    y_sb = ypool.tile([P, 4, TB], BF16, tag="ysb", name=f"y{bk}_{ts_}")
                    for e in range(4):
                        y_ps = pss.tile([P, TB], F32, tag="work", bufs=2, name=f"yp{bk}_{ts_}_{e}")
                        for h in range(NREP):
                            nc.tensor.matmul(y_ps, oh_l[bk][h][:, ts_ * P:(ts_ + 1) * P],
                                             wo_sb[:, h, e * TB:(e + 1) * TB],
                                             start=(h == 0), stop=(h == NREP - 1))
                        nc.vector.tensor_copy(y_sb[:, e], y_ps)
                    nc.sync.dma_start(
                        out=out[t0 + ts_ * P:t0 + (ts_ + 1) * P, :], in_=y_sb)

            # emission order = desired PE order (keeps PE dense / HAM warm)
            proj(0)
            proj(1)
            attn(0)
            proj(2)
            attn(1)
            wo_stage(0)
            proj(3)
            attn(2)
            wo_stage(1)
            attn(3)
            wo_stage(2)
            wo_stage(3)

    nc.compile()
    return nc


def _perm():
    """RoPE pair permutation: partner pairs (i, i+64) are placed 16 apart
    within the same 32-partition block so the DVE stream_shuffle (which only
    shuffles within 32-partition blocks) can do the partner swap."""
    perm = np.empty(HD, np.int64)
    for b in range(4):
        for s in range(16):
            perm[32 * b + s] = 16 * b + s
            perm[32 * b + 16 + s] = 64 + 16 * b + s
    newpos = np.arange(HD)
    sign_new = np.where(newpos % 32 < 16, -1.0, 1.0).astype(np.float32)
    return perm, sign_new


def _prep_inputs(x, mask, freqs_cis, alibi_bias, Wq, Wk, Wv, Wo):
    """Host-side prep: transposes, RoPE tables, ALiBi bias decomposition."""
    f64 = np.float64
    perm, sign_new = _perm()
    idx = np.arange(HD)
    cos_full = freqs_cis[:, idx // 2]                     # [T, 128]
    sin_full = freqs_cis[:, (HD // 2) + idx // 2]         # [T, 128]
    cosT = np.ascontiguousarray(cos_full.T[perm])         # [128, T], permuted
    sinT_signed = np.ascontiguousarray(sin_full.T[perm] * sign_new[:, None])

    cosq = (cosT * np.float32(SCALE)).astype(BFNP)
    sinq = (sinT_signed * np.float32(SCALE)).astype(BFNP)
    cosk = cosT.astype(BFNP)
    sink = sinT_signed.astype(BFNP)

    # permute rope feature columns of Wq (per q-head 128-block) and Wk
    Wq_p = Wq.reshape(D, H, HD)[:, :, perm].reshape(D, D)
    Wk_p = Wk.reshape(D, KV, HD)[:, :, perm].reshape(D, KV * HD)

    # multiplicative causal mask for the 4 diagonal key-tile offsets:
    # m01[p, delta*TB + f] = 1 if (128*delta + p) <= f else 0
    pvec = np.arange(P)[:, None]
    fvec = np.arange(TB)[None, :]
    m01 = np.empty((P, 4 * TB), np.float32)
    for delta in range(4):
        m01[:, delta * TB:(delta + 1) * TB] = (128 * delta + pvec <= fvec)
    m01 = m01.astype(BFNP)

    onesc = np.ones((P, 1), np.float32).astype(BFNP)
    idin = np.eye(P, dtype=np.float32).astype(BFNP)

    in_maps = []
    for c in range(8):
        b, g = c // 4, c % 4
        slopes = np.array([-f64(alibi_bias[0, g * NREP + hl, 1, 0]) for hl in range(NREP)])
        pvec64 = np.arange(P, dtype=f64)
        jvec = np.arange(NJ, dtype=f64)
        # cb[p, h, bk, j] = ALIBI_W*slope*(j*128 + p) - ALIBI_W*slope*(bk*512 + 511)
        bkvec = np.arange(NBLK, dtype=f64)
        cbv = (ALIBI_W * slopes[:, None, None, None]
               * (jvec[None, None, :, None] * P + pvec64[None, None, None, :]
                  - (bkvec[None, :, None, None] * TB + (TB - 1))))
        cbm = np.ascontiguousarray(cbv.transpose(3, 0, 1, 2).reshape(P, NREP * NBLK * NJ)).astype(np.float32)
        in_maps.append({
            "xT": np.ascontiguousarray(x[b].T).astype(BFNP),
            "wq": np.ascontiguousarray(Wq_p[:, g * KVD:(g + 1) * KVD]).astype(BFNP),
            "wk": np.ascontiguousarray(Wk_p[:, g * P:(g + 1) * P]).astype(BFNP),
            "wv": np.ascontiguousarray(Wv[:, g * P:(g + 1) * P]).astype(BFNP),
            "wo": np.ascontiguousarray(Wo[g * KVD:(g + 1) * KVD, :]).astype(BFNP),
            "cosq": cosq, "sinq": sinq, "cosk": cosk, "sink": sink,
            "cb": cbm, "m01": m01,
            "onesc": onesc, "idin": idin,
        })
    return in_maps


def kernel(x, mask, freqs_cis, alibi_bias, Wq, Wk, Wv, Wo, _trace=False, _trace_kwargs=None):
    from concourse.bass_utils import run_bass_kernel_spmd

    if "nc" not in _cache:
        _cache["nc"] = _build()
    nc = _cache["nc"]

    in_maps = _prep_inputs(np.asarray(x, np.float32), np.asarray(mask, np.float32),
                           np.asarray(freqs_cis, np.float32), np.asarray(alibi_bias, np.float32),
                           np.asarray(Wq, np.float32), np.asarray(Wk, np.float32),
                           np.asarray(Wv, np.float32), np.asarray(Wo, np.float32))
    kw = {}
    if _trace:
        kw = dict(trace=True, **(_trace_kwargs or {}))
    res = run_bass_kernel_spmd(nc, in_maps, list(range(8)), **kw)

    full = np.zeros((B, T, D), np.float32)
    for c in range(8):
        full[c // 4] += np.asarray(res.results[c]["out"], np.float32)
    if _trace:
        _cache["last_trace"] = res
    return full


# revision 55
# speedup vs baseline: 1.0433x; 1.0433x over previous
"""GQA attention (RoPE + ALiBi + causal) on 8 trn2 NeuronCores.

Sharding: core c -> batch b = c//4, kv-group g = c%4 (4 q-heads + 1 kv-head
per core, column-sharded Wq/Wk/Wv, row-sharded Wo; host sums the 4 partial
Wo outputs per batch).

All matmuls run in bf16 (f32 PSUM accumulation). Everything is kept
transposed ([feature, token]) so softmax reductions over keys become
partition-dim reductions done with ones-vector matmuls, and the per-key
ALiBi column bias rides the exp() activation's per-partition bias (the
per-query ALiBi part is constant per token and cancels in softmax up to
the per-block offset baked into the bias). Causal structure: only
lower-triangle key tiles are computed; diagonal tiles are masked with a
multiplicative {0,1} mask after exp. V is projected directly transposed
(x-tile stationary). The emission order interleaves next-block
projections between attention and Wo so the PE never idles long enough
for HAM to re-throttle.
"""
import sys

if '/opt/trn_rl_repo' not in sys.path:
    sys.path.insert(0, '/opt/trn_rl_repo')

import numpy as np
import ml_dtypes

BFNP = ml_dtypes.bfloat16

B, T, D = 2, 2048, 2048
H, KV = 16, 4
HD = D // H          # 128
NREP = H // KV       # 4
KVD = 512            # per-core q width (4 heads x 128)
P = 128
TB = 512             # t-block
NBLK = T // TB       # 4
NC = D // P          # 16 contraction tiles
NJ = T // P          # 16 key tiles
ALIBI_W = 0.1
SCALE = (1.0 - ALIBI_W) / np.sqrt(np.float32(HD))

# stream_shuffle: rotate partitions by 64 (16 groups of 4) - self-inverse
SHUF_MASK = [(g + 16) % 32 for g in range(32)]

_cache = {}


def _build():
    from concourse import bacc, mybir, bass_isa
    from concourse.tile import TileContext

    F32 = mybir.dt.float32
    BF16 = mybir.dt.bfloat16
    EXP = mybir.ActivationFunctionType.Exp

    nc = bacc.Bacc()
    xT = nc.declare_dram_parameter("xT", [D, T], BF16, isOutput=False)
    wq = nc.declare_dram_parameter("wq", [D, KVD], BF16, isOutput=False)
    wk = nc.declare_dram_parameter("wk", [D, P], BF16, isOutput=False)
    wv = nc.declare_dram_parameter("wv", [D, P], BF16, isOutput=False)
    wo = nc.declare_dram_parameter("wo", [KVD, D], BF16, isOutput=False)
    cosq = nc.declare_dram_parameter("cosq", [P, T], BF16, isOutput=False)
    sinq = nc.declare_dram_parameter("sinq", [P, T], BF16, isOutput=False)
    cosk = nc.declare_dram_parameter("cosk", [P, T], BF16, isOutput=False)
    sink = nc.declare_dram_parameter("sink", [P, T], BF16, isOutput=False)
    cb = nc.declare_dram_parameter("cb", [P, NREP * NBLK * NJ], F32, isOutput=False)
    m01 = nc.declare_dram_parameter("m01", [P, 4 * TB], BF16, isOutput=False)
    onesc = nc.declare_dram_parameter("onesc", [P, 1], BF16, isOutput=False)
    idin = nc.declare_dram_parameter("idin", [P, P], BF16, isOutput=False)
    out = nc.declare_dram_parameter("out", [T, D], BF16, isOutput=True)

    with TileContext(nc) as tc:
        with (
            tc.tile_pool(name="const", bufs=1) as cpool,
            tc.tile_pool(name="kv", bufs=1) as kvpool,
            tc.tile_pool(name="xin", bufs=2) as xpool,
            tc.tile_pool(name="work", bufs=2) as wpool,
            tc.tile_pool(name="qt", bufs=8) as qpool,
            tc.tile_pool(name="pt", bufs=6) as ptpool,
            tc.tile_pool(name="ot", bufs=8) as opool,
            tc.tile_pool(name="ysb", bufs=2) as ypool,
            tc.tile_pool(name="small", bufs=2) as spool,
            tc.tile_pool(name="ps", bufs=1, space="PSUM") as pss,
        ):
            # ---- resident constants ----
            # DMA priority: x block 0 + Wq first (needed by the first matmuls),
            # then the rest in order of first use.
            xTr = xT.rearrange("(c p) t -> p c t", p=P)
            xt0 = xpool.tile([P, NC, TB], BF16, tag="xt", name="xt0")
            wq_sb = cpool.tile([P, NC, KVD], BF16)
            wq_r = wq.rearrange("(c p) n -> p c n", p=P)
            # interleave small x/Wq chunks so the first q-wave matmuls can
            # start as soon as the first chunks land; spread the issue cost
            # across engines (each dma_start costs ~0.6us on its issuing
            # engine, all of which are idle during the prologue)
            for c2 in range(8):
                nc.sync.dma_start(out=xt0[:, 2 * c2:2 * c2 + 2],
                                  in_=xTr[:, 2 * c2:2 * c2 + 2, 0:TB])
                nc.sync.dma_start(out=wq_sb[:, 2 * c2:2 * c2 + 2],
                                  in_=wq_r[:, 2 * c2:2 * c2 + 2])
            wk_sb = cpool.tile([P, NC, P], BF16)
            wk_r = wk.rearrange("(c p) n -> p c n", p=P)
            nc.sync.dma_start(out=wk_sb, in_=wk_r)
            wv_sb = cpool.tile([P, NC, P], BF16)
            wv_r = wv.rearrange("(c p) n -> p c n", p=P)
            nc.sync.dma_start(out=wv_sb, in_=wv_r)
            cq_sb = cpool.tile([P, T], BF16)
            nc.sync.dma_start(out=cq_sb, in_=cosq[:, :])
            sq_sb = cpool.tile([P, T], BF16)
            nc.sync.dma_start(out=sq_sb, in_=sinq[:, :])
            ck_sb = cpool.tile([P, T], BF16)
            nc.sync.dma_start(out=ck_sb, in_=cosk[:, :])
            sk_sb = cpool.tile([P, T], BF16)
            nc.sync.dma_start(out=sk_sb, in_=sink[:, :])
            cb_sb = cpool.tile([P, NREP * NBLK * NJ], F32)
            nc.sync.dma_start(out=cb_sb, in_=cb[:, :])
            m01_sb = cpool.tile([P, 4 * TB], BF16)
            nc.sync.dma_start(out=m01_sb, in_=m01[:, :])
            ones_sb = cpool.tile([P, 1], BF16)
            nc.sync.dma_start(out=ones_sb, in_=onesc[:, :])
            id_sb = cpool.tile([P, P], BF16)
            nc.sync.dma_start(out=id_sb, in_=idin[:, :])
            wo_sb = cpool.tile([P, NREP, D], BF16)
            wo_r = wo.rearrange("(h p) e -> p h e", p=P)
            for h in range(NREP):
                nc.sync.dma_start(out=wo_sb[:, h], in_=wo_r[:, h])

            kT_sb = kvpool.tile([P, T], BF16)        # roped K, [d, s]
            v_sb = kvpool.tile([P, NJ * P], BF16)    # V tiles, [s, j*d']

            qh_l = [[None] * NREP for _ in range(NBLK)]
            oh_l = [[None] * NREP for _ in range(NBLK)]

            def rope(dst, src_ps, cos_ap, sin_ap, nm):
                # muls/adds on GpSimd (otherwise idle) to unload the DVE;
                # the cross-partition shuffle must stay on the DVE
                raw = wpool.tile([P, TB], BF16, tag="raw", name=f"raw{nm}")
                nc.scalar.copy(raw, src_ps)
                swp = wpool.tile([P, TB], BF16, tag="swp", name=f"swp{nm}")
                nc.vector.stream_shuffle(swp, raw, SHUF_MASK)
                m1 = wpool.tile([P, TB], BF16, tag="m1", name=f"m1{nm}")
                nc.vector.tensor_mul(m1, raw, cos_ap)
                m2 = wpool.tile([P, TB], BF16, tag="m2", name=f"m2{nm}")
                nc.vector.tensor_mul(m2, swp, sin_ap)
                nc.vector.tensor_add(dst, m1, m2)

            def proj(bk):
                t0 = bk * TB
                if bk == 0:
                    xt = xt0
                else:
                    xt = xpool.tile([P, NC, TB], BF16, tag="xt", name=f"xt{bk}")
                    nc.sync.dma_start(out=xt[:, 0:8], in_=xTr[:, 0:8, t0:t0 + TB])
                    nc.sync.dma_start(out=xt[:, 8:16], in_=xTr[:, 8:16, t0:t0 + TB])
                # q waves (one PSUM tile each)
                for h in range(NREP):
                    qp = pss.tile([P, TB], F32, tag="work", bufs=2, name=f"qp{bk}_{h}")
                    for c in range(NC):
                        nc.tensor.matmul(qp, wq_sb[:, c, h * P:(h + 1) * P], xt[:, c],
                                         start=(c == 0), stop=(c == NC - 1))
                    qh = qpool.tile([P, TB], BF16, tag="qh", name=f"qh{bk}_{h}")
                    rope(qh, qp, cq_sb[:, t0:t0 + TB], sq_sb[:, t0:t0 + TB], f"q{bk}{h}")
                    qh_l[bk][h] = qh
                # k wave
                kp = pss.tile([P, TB], F32, tag="work", bufs=2, name=f"kp{bk}")
                for c in range(NC):
                    nc.tensor.matmul(kp, wk_sb[:, c], xt[:, c], start=(c == 0), stop=(c == NC - 1))
                rope(kT_sb[:, t0:t0 + TB], kp, ck_sb[:, t0:t0 + TB], sk_sb[:, t0:t0 + TB], f"k{bk}")
                # v wave: directly transposed ([tok, d']) via x-tile-stationary
                # matmuls; one PSUM tile per token slice (a start=True matmul
                # clears has_written for its whole bank, so accumulation groups
                # must not share a bank). Token slices are processed in pairs
                # with the matmuls interleaved across the two banks so each
                # matmul's drain overlaps the other's fill.
                for pair in range(2):
                    vtps = [pss.tile([P, P], F32, tag="work", bufs=2, name=f"vtp{bk}_{2 * pair + i}")
                            for i in range(2)]
                    for c in range(NC):
                        for i in range(2):
                            ts_ = 2 * pair + i
                            nc.tensor.matmul(vtps[i], xt[:, c, ts_ * P:(ts_ + 1) * P],
                                             wv_sb[:, c], start=(c == 0), stop=(c == NC - 1))
                    for i in range(2):
                        ts_ = 2 * pair + i
                        nc.vector.tensor_copy(
                            v_sb[:, (4 * bk + ts_) * P:(4 * bk + ts_ + 1) * P], vtps[i])

            def attn(bk):
                nj = 4 * bk + 4
                for h in range(NREP):
                    ot_ps = pss.tile([P, TB], F32, tag="ot", bufs=2, name=f"ot{bk}_{h}")
                    acc = spool.tile([P, TB], BF16, tag="acc", name=f"acc{bk}_{h}")

                    def post(sp, j):
                        # diag key tile delta: tokens < 128*delta never attend
                        # to these keys -> compute only the [128*delta, TB)
                        # token range; the triangular mask only applies to the
                        # first 128 tokens of that range (the corner)
                        delta = j - 4 * bk
                        d0 = max(delta, 0) * P
                        col = (h * NBLK + bk) * NJ + j
                        pt = ptpool.tile([P, TB], BF16, tag="pt", name=f"pt{bk}_{h}_{j}")
                        nc.scalar.activation(pt[:, d0:TB], sp[:, d0:TB], EXP,
                                             bias=cb_sb[:, col:col + 1])
                        if delta >= 0:
                            # mask the 128-wide triangular corner in place
                            nc.vector.tensor_mul(pt[:, d0:d0 + P], pt[:, d0:d0 + P],
                                                 m01_sb[:, 0:P])
                        if j == 0:
                            nc.vector.tensor_copy(acc, pt)
                        else:
                            nc.vector.tensor_add(acc[:, d0:TB], acc[:, d0:TB], pt[:, d0:TB])
                        nc.tensor.matmul(ot_ps[:, d0:TB], v_sb[:, j * P:(j + 1) * P],
                                         pt[:, d0:TB], start=(j == 0), stop=(j == nj - 1))

                    pend = None
                    for j in range(nj):
                        sp = pss.tile([P, TB], F32, tag="s", bufs=3, name=f"sp{bk}_{h}_{j}")
                        d0 = max(j - 4 * bk, 0) * P
                        nc.tensor.matmul(sp[:, d0:TB], kT_sb[:, j * P:(j + 1) * P],
                                         qh_l[bk][h][:, d0:TB], start=True, stop=True)
                        if pend is not None:
                            post(*pend)
                        pend = (sp, j)
                    post(*pend)

                    cs_ps = pss.tile([1, TB], F32, tag="cs", bufs=1, name=f"cs{bk}_{h}")
                    nc.tensor.matmul(cs_ps, ones_sb, acc, start=True, stop=True)
                    rin = spool.tile([1, TB], F32, tag="rin", name=f"rin{bk}_{h}")
                    nc.vector.reciprocal_approx_fast(out=rin, in_=cs_ps)
                    rbc = spool.tile([P, TB], F32, tag="rbc", name=f"rbc{bk}_{h}")
                    nc.gpsimd.partition_broadcast(rbc, rin)
                    oh = opool.tile([P, TB], BF16, tag="oh", name=f"oh{bk}_{h}")
                    nc.vector.tensor_mul(oh, ot_ps, rbc)
                    oh_l[bk][h] = oh

            def wo_stage(bk):
                t0 = bk * TB
                for ts_ in range(4):
                    y_sb = ypool.tile([P, 4, TB], BF16, tag="ysb", name=f"y{bk}_{ts_}")
                    for e in range(4):
                        y_ps = pss.tile([P, TB], F32, tag="work", bufs=2, name=f"yp{bk}_{ts_}_{e}")
                        for h in range(NREP):
                            nc.tensor.matmul(y_ps, oh_l[bk][h][:, ts_ * P:(ts_ + 1) * P],
                                             wo_sb[:, h, e * TB:(e + 1) * TB],
                                             start=(h == 0), stop=(h == NREP - 1))
                        nc.vector.tensor_copy(y_sb[:, e], y_ps)
                    nc.sync.dma_start(
                        out=out[t0 + ts_ * P:t0 + (ts_ + 1) * P, :], in_=y_sb)

            # emission order = desired PE order (keeps PE dense / HAM warm)
            proj(0)
            proj(1)
            attn(0)
            proj(2)
            attn(1)
            wo_stage(0)
            proj(3)
            attn(2)
            wo_stage(1)
            attn(3)
            wo_stage(2)
            wo_stage(3)

    nc.compile()
    return nc


def _perm():
    """RoPE pair permutation: partner pairs (i, i+64) are placed 16 apart
    within the same 32-partition block so the DVE stream_shuffle (which only
    shuffles within 32-partition blocks) can do the partner swap."""
    perm = np.empty(HD, np.int64)
    for b in range(4):
        for s in range(16):
            perm[32 * b + s] = 16 * b + s
            perm[32 * b + 16 + s] = 64 + 16 * b + s
    newpos = np.arange(HD)
    sign_new = np.where(newpos % 32 < 16, -1.0, 1.0).astype(np.float32)
    return perm, sign_new


def _prep_inputs(x, mask, freqs_cis, alibi_bias, Wq, Wk, Wv, Wo):
    """Host-side prep: transposes, RoPE tables, ALiBi bias decomposition."""
    f64 = np.float64
    perm, sign_new = _perm()
    idx = np.arange(HD)
    cos_full = freqs_cis[:, idx // 2]                     # [T, 128]
    sin_full = freqs_cis[:, (HD // 2) + idx // 2]         # [T, 128]
    cosT = np.ascontiguousarray(cos_full.T[perm])         # [128, T], permuted
    sinT_signed = np.ascontiguousarray(sin_full.T[perm] * sign_new[:, None])

    cosq = (cosT * np.float32(SCALE)).astype(BFNP)
    sinq = (sinT_signed * np.float32(SCALE)).astype(BFNP)
    cosk = cosT.astype(BFNP)
    sink = sinT_signed.astype(BFNP)

    # permute rope feature columns of Wq (per q-head 128-block) and Wk
    Wq_p = Wq.reshape(D, H, HD)[:, :, perm].reshape(D, D)
    Wk_p = Wk.reshape(D, KV, HD)[:, :, perm].reshape(D, KV * HD)

    # multiplicative causal mask for the 4 diagonal key-tile offsets:
    # m01[p, delta*TB + f] = 1 if (128*delta + p) <= f else 0
    pvec = np.arange(P)[:, None]
    fvec = np.arange(TB)[None, :]
    m01 = np.empty((P, 4 * TB), np.float32)
    for delta in range(4):
        m01[:, delta * TB:(delta + 1) * TB] = (128 * delta + pvec <= fvec)
    m01 = m01.astype(BFNP)

    onesc = np.ones((P, 1), np.float32).astype(BFNP)
    idin = np.eye(P, dtype=np.float32).astype(BFNP)

    in_maps = []
    for c in range(8):
        b, g = c // 4, c % 4
        slopes = np.array([-f64(alibi_bias[0, g * NREP + hl, 1, 0]) for hl in range(NREP)])
        pvec64 = np.arange(P, dtype=f64)
        jvec = np.arange(NJ, dtype=f64)
        # cb[p, h, bk, j] = ALIBI_W*slope*(j*128 + p) - ALIBI_W*slope*(bk*512 + 511)
        bkvec = np.arange(NBLK, dtype=f64)
        cbv = (ALIBI_W * slopes[:, None, None, None]
               * (jvec[None, None, :, None] * P + pvec64[None, None, None, :]
                  - (bkvec[None, :, None, None] * TB + (TB - 1))))
        cbm = np.ascontiguousarray(cbv.transpose(3, 0, 1, 2).reshape(P, NREP * NBLK * NJ)).astype(np.float32)
        in_maps.append({
            "xT": np.ascontiguousarray(x[b].T).astype(BFNP),
            "wq": np.ascontiguousarray(Wq_p[:, g * KVD:(g + 1) * KVD]).astype(BFNP),
            "wk": np.ascontiguousarray(Wk_p[:, g * P:(g + 1) * P]).astype(BFNP),
            "wv": np.ascontiguousarray(Wv[:, g * P:(g + 1) * P]).astype(BFNP),
            "wo": np.ascontiguousarray(Wo[g * KVD:(g + 1) * KVD, :]).astype(BFNP),
            "cosq": cosq, "sinq": sinq, "cosk": cosk, "sink": sink,
            "cb": cbm, "m01": m01,
            "onesc": onesc, "idin": idin,
        })
    return in_maps


def kernel(x, mask, freqs_cis, alibi_bias, Wq, Wk, Wv, Wo, _trace=False, _trace_kwargs=None):
    from concourse.bass_utils import run_bass_kernel_spmd

    if "nc" not in _cache:
        _cache["nc"] = _build()
    nc = _cache["nc"]

    in_maps = _prep_inputs(np.asarray(x, np.float32), np.asarray(mask, np.float32),
                           np.asarray(freqs_cis, np.float32), np.asarray(alibi_bias, np.float32),
                           np.asarray(Wq, np.float32), np.asarray(Wk, np.float32),
                           np.asarray(Wv, np.float32), np.asarray(Wo, np.float32))
    kw = {}
    if _trace:
        kw = dict(trace=True, **(_trace_kwargs or {}))
    res = run_bass_kernel_spmd(nc, in_maps, list(range(8)), **kw)

    full = np.zeros((B, T, D), np.float32)
    for c in range(8):
        full[c // 4] += np.asarray(res.results[c]["out"], np.float32)
    if _trace:
        _cache["last_trace"] = res
    return full
